# revision 2
# baseline (speedup 1.0000x reference)
"""Trainium2 Bass kernel for CrossAttention (b=4, p=8, n=512, dim=512, 8 heads x 64).

Sharding: the 32 independent (b, p) slices are split 4-per-core across 8
NeuronCores (pure data parallel, no collectives). Weights are replicated.

All per-core device inputs are packed into ONE bf16 DRAM tensor (the
axon-tunneled dispatch path charges ~30us/call per operand, so operand
count dominates the amortized per-call time):
  [ qT(4*512*512) | kvT(4*512*512) | Wq | Wk | Wv | Wo (512*512 each) | bo(512) ]
Activations are pre-transposed per-slice to [dim, n] on the host.

Device dataflow is software-pipelined across slices: slice s+1's
projection matmul blocks are emitted between slice s's attention head
pairs, so the PE queue always has ready work while the attention chain
waits on ACT exps / DVE normalizes.

Per-slice structure (all SBUF tiles are [partition, free]):
  - qT = Wq-blocks^T @ xqT, kT likewise; v = xkvT-blocks^T @ Wv  (PE)
  - ST per head pair, jb-interleaved: h0 (PE row groups 0-1) and h1
    (row groups 2-3) issue back-to-back and run concurrently.
    ST[j, i] = kT_h-block^T @ qT_h -> exp (ACT, scale=1/8) -> PT bf16
    (no max subtraction: scores are ~N(0,1), exp cannot overflow)
  - PV column-group-paired: even head outT -> PSUM[0:64] while the odd
    head's l row-sum matmul (ones) lands in PSUM[64:128] of a second
    bank, and vice versa -> both PE issues run concurrently.
  - normalize: one [128,512] reciprocal + multiply per head pair.
  - final: fin = outT-blocks^T @ Wo; bias added by the DVE evacuation
    (tensor add with pre-broadcast bo); per-row-block DMA to DRAM.
"""

from contextlib import ExitStack

import ml_dtypes
import numpy as np

import concourse.bass as bass
import concourse.tile as tile
from concourse import bacc, mybir
from concourse.bass_utils import run_bass_kernel_spmd

F32 = mybir.dt.float32
BF16 = mybir.dt.bfloat16

HEADS = 8
DH = 64
N = 512
DIM = 512
SCALE = DH**-0.5
S = 4  # (b, p) slices per core
N_CORES = 8

# packed input offsets (bf16 elements)
SZ_X = S * DIM * N  # 1048576
SZ_W = DIM * DIM  # 262144
OFF_QT = 0
OFF_KVT = SZ_X
OFF_W = 2 * SZ_X
OFF_BO = 2 * SZ_X + 4 * SZ_W
TOT = OFF_BO + DIM


def _build_body(ctx: ExitStack, tc: tile.TileContext, xin, out):
    nc = tc.nc

    const = ctx.enter_context(tc.tile_pool(name="const", bufs=1))
    xT = ctx.enter_context(tc.tile_pool(name="xT", bufs=4))
    proj = ctx.enter_context(tc.tile_pool(name="proj", bufs=2))
    ptp = ctx.enter_context(tc.tile_pool(name="ptp", bufs=4))
    outTp = ctx.enter_context(tc.tile_pool(name="outTp", bufs=2))
    rbp = ctx.enter_context(tc.tile_pool(name="rbp", bufs=2))
    finp = ctx.enter_context(tc.tile_pool(name="finp", bufs=2))
    mm_ps = ctx.enter_context(tc.tile_pool(name="mm_ps", bufs=2, space="PSUM"))
    st_ps = ctx.enter_context(tc.tile_pool(name="st_ps", bufs=3, space="PSUM"))
    pv_ps = ctx.enter_context(tc.tile_pool(name="pv_ps", bufs=2, space="PSUM"))
    l_ps = ctx.enter_context(tc.tile_pool(name="l_ps", bufs=1, space="PSUM"))

    # chunked loads (one [128, 512] chunk per contraction block) so the
    # first matmul's dependency clears after ~2 chunks, not 2 full tiles
    def load_chunked(dst, off):
        for d in range(4):
            nc.sync.dma_start(
                dst[:, d * 512 : (d + 1) * 512],
                xin[off + d * 128 * 512 : off + (d + 1) * 128 * 512].rearrange(
                    "(p n) -> p n", p=128
                ),
            )

    # --- weights + early inputs ---
    wq16 = const.tile([128, 4 * 512], BF16, name="wq16")
    load_chunked(wq16, OFF_W)
    xq = [None] * S
    xkv = [None] * S
    xq[0] = xT.tile([128, 4 * 512], BF16, name="xqT", tag="xq")
    load_chunked(xq[0], OFF_QT)
    wk16 = const.tile([128, 4 * 512], BF16, name="wk16")
    load_chunked(wk16, OFF_W + SZ_W)
    xkv[0] = xT.tile([128, 4 * 512], BF16, name="xkvT", tag="xkv")
    load_chunked(xkv[0], OFF_KVT)
    wv16 = const.tile([128, 4 * 512], BF16, name="wv16")
    load_chunked(wv16, OFF_W + 2 * SZ_W)
    wo16 = const.tile([128, 4 * 512], BF16, name="wo16")
    load_chunked(wo16, OFF_W + 3 * SZ_W)
    bo16 = const.tile([1, 512], BF16, name="bo16")
    nc.sync.dma_start(bo16[:], xin[OFF_BO : OFF_BO + 512].rearrange("(o f) -> o f", o=1))
    ones64 = const.tile([128, 64], BF16, name="ones64")
    nc.gpsimd.memset(ones64[:], 1.0)
    ones1 = const.tile([1, 128], BF16, name="ones1")
    nc.gpsimd.memset(ones1[:], 1.0)
    bo_bc = const.tile([128, 512], F32, name="bo_bc")

    def load_inputs(s):
        xq[s] = xT.tile([128, 4 * 512], BF16, name="xqT", tag="xq")
        load_chunked(xq[s], OFF_QT + s * DIM * N)
        xkv[s] = xT.tile([128, 4 * 512], BF16, name="xkvT", tag="xkv")
        load_chunked(xkv[s], OFF_KVT + s * DIM * N)

    def alloc_proj(s):
        qT16 = proj.tile([128, 4 * 512], BF16, name="qT16", tag="qT", bufs=2)
        kT16 = proj.tile([128, 4 * 512], BF16, name="kT16", tag="kT", bufs=2)
        v16 = proj.tile([128, 4 * 512], BF16, name="v16", tag="v", bufs=2)
        return qT16, kT16, v16

    def proj_blocks(s, tiles):
        """12 closures: (q-t, k-t, v-jb) round-robin over t/jb = 0..3."""
        qT16, kT16, v16 = tiles
        blocks = []

        def qk_block(w16, xt, dst, t):
            def emit():
                ps = mm_ps.tile([128, 512], F32, name="mm_ps")
                for d in range(4):
                    nc.tensor.matmul(
                        ps[:],
                        w16[:, d * 512 + t * 128 : d * 512 + (t + 1) * 128],
                        xt[:, d * 512 : (d + 1) * 512],
                        start=(d == 0),
                        stop=(d == 3),
                    )
                nc.vector.tensor_copy(dst[:, t * 512 : (t + 1) * 512], ps[:])

            return emit

        def v_block(jb):
            def emit():
                ps = mm_ps.tile([128, 512], F32, name="mm_ps")
                for d in range(4):
                    nc.tensor.matmul(
                        ps[:],
                        xkv[s][:, d * 512 + jb * 128 : d * 512 + (jb + 1) * 128],
                        wv16[:, d * 512 : (d + 1) * 512],
                        start=(d == 0),
                        stop=(d == 3),
                    )
                nc.vector.tensor_copy(v16[:, jb * 512 : (jb + 1) * 512], ps[:])

            return emit

        for t in range(4):
            blocks.append(qk_block(wq16, xq[s], qT16, t))
            blocks.append(qk_block(wk16, xkv[s], kT16, t))
            blocks.append(v_block(t))
        return blocks

    # --- prologue: slice 0 (and slice 1 inputs) ---
    load_inputs(1)
    cur = alloc_proj(0)
    for blk in proj_blocks(0, cur):
        blk()

    for s in range(S):
        qT16, kT16, v16 = cur
        if s + 2 < S:
            load_inputs(s + 2)
        if s + 1 < S:
            nxt = alloc_proj(s + 1)
            nxt_blocks = proj_blocks(s + 1, nxt)
        else:
            nxt, nxt_blocks = None, []

        outT16 = outTp.tile([128, 4 * 512], BF16, name="outT16")
        for tp in range(4):
            h0, h1 = 2 * tp, 2 * tp + 1
            # ST, jb-interleaved across the head pair: h0 uses PE row
            # groups 0-1, h1 row groups 2-3 -> adjacent issues overlap.
            pt_e = ptp.tile([128, 4 * 512], BF16, name="pt16", tag="pte", bufs=2)
            pt_o = ptp.tile([128, 4 * 512], BF16, name="pt16o", tag="pto", bufs=2)
            for jb in range(4):
                for half, pt16 in ((0, pt_e), (64, pt_o)):
                    kT_h = kT16[half : half + 64, tp * 512 : (tp + 1) * 512]
                    qT_h = qT16[half : half + 64, tp * 512 : (tp + 1) * 512]
                    stt = st_ps.tile([128, 512], F32, name="st_ps")
                    nc.tensor.matmul(
                        stt[:],
                        kT_h[:, jb * 128 : (jb + 1) * 128],
                        qT_h,
                        start=True,
                        stop=True,
                    )
                    nc.scalar.activation(
                        pt16[:, jb * 512 : (jb + 1) * 512],
                        stt[:],
                        mybir.ActivationFunctionType.Exp,
                        scale=SCALE,
                    )

            # PV: column-group-paired issues so PE overlaps outT with l.
            pv = pv_ps.tile([128, 512], F32, name="pv_ps")
            lps = l_ps.tile([128, 512], F32, name="l_ps")
            for jb in range(4):
                pe_s = pt_e[:, jb * 512 : (jb + 1) * 512]
                po_s = pt_o[:, jb * 512 : (jb + 1) * 512]
                st, sp = (jb == 0), (jb == 3)
                # issue A: even outT (cols 0-63) || odd l-bcast (cols 64-127)
                nc.tensor.matmul(
                    pv[0:64, :],
                    v16[:, jb * 512 + h0 * 64 : jb * 512 + (h0 + 1) * 64],
                    pe_s, start=st, stop=sp, skip_group_check=True,
                )
                nc.tensor.matmul(
                    lps[64:128, :], ones64[:], po_s, start=st, stop=sp,
                    skip_group_check=True,
                )
                # issue B: odd outT (cols 64-127) || even l-bcast (cols 0-63)
                nc.tensor.matmul(
                    pv[64:128, :],
                    v16[:, jb * 512 + h1 * 64 : jb * 512 + (h1 + 1) * 64],
                    po_s, start=st, stop=sp, skip_group_check=True,
                )
                nc.tensor.matmul(
                    lps[0:64, :], ones64[:], pe_s, start=st, stop=sp,
                    skip_group_check=True,
                )
            # normalize both heads at once: lps holds l_h0 bcast on
            # partitions 0-63 and l_h1 on 64-127, matching pv's layout.
            rb1 = rbp.tile([128, 512], F32, name="rb1")
            nc.vector.reciprocal(rb1[:], lps[:])
            nc.vector.tensor_mul(
                outT16[:, tp * 512 : (tp + 1) * 512], pv[:], rb1[:]
            )

            # keep the PE queue fed: 3 of slice s+1's projection blocks
            for blk in nxt_blocks[3 * tp : 3 * tp + 3]:
                blk()

        # --- final projection; bias via the DVE evacuation add ---
        if s == 0:
            # broadcast bo across all 128 partitions (issued late so the
            # PE queue doesn't head-block on the bo DMA at startup)
            bops = mm_ps.tile([128, 512], F32, name="mm_ps")
            nc.tensor.matmul(bops[:], ones1[:], bo16[:], start=True, stop=True)
            nc.vector.tensor_copy(bo_bc[:], bops[:])
        fin = finp.tile([128, 4 * 512], F32, name="fin")
        for ib in range(4):
            ps = mm_ps.tile([128, 512], F32, name="mm_ps")
            for t in range(4):
                nc.tensor.matmul(
                    ps[:],
                    outT16[:, t * 512 + ib * 128 : t * 512 + (ib + 1) * 128],
                    wo16[:, t * 512 : (t + 1) * 512],
                    start=(t == 0),
                    stop=(t == 3),
                )
            nc.vector.tensor_add(fin[:, ib * 512 : (ib + 1) * 512], ps[:], bo_bc[:])
            # store each row-block as soon as its evacuation lands
            nc.sync.dma_start(
                out[s][ib * 128 : (ib + 1) * 128, :],
                fin[:, ib * 512 : (ib + 1) * 512],
            )
        cur = nxt


def build_nc():
    nc = bacc.Bacc(
        "TRN2", target_bir_lowering=False, debug=False, enable_partition_id=False
    )
    xin = nc.dram_tensor("xin", [TOT], BF16, kind="ExternalInput").ap()
    out = nc.dram_tensor("out", [S, N, DIM], F32, kind="ExternalOutput").ap()
    with tile.TileContext(nc) as tc:
        with ExitStack() as ctx:
            _build_body(ctx, tc, xin, out)
    nc.compile()
    return nc


_NC = None
BF = ml_dtypes.bfloat16


def make_in_maps(q_in, kv_in, Wq, Wk, Wv, Wo, bo):
    # host-side layout prep: per-slice transpose to [dim, n] + bf16 cast,
    # then pack everything into one flat bf16 tensor per core.
    q = np.asarray(q_in, dtype=np.float32).reshape(32, N, DIM)
    kv = np.asarray(kv_in, dtype=np.float32).reshape(32, N, DIM)
    qT = np.ascontiguousarray(q.transpose(0, 2, 1)).astype(BF)
    kvT = np.ascontiguousarray(kv.transpose(0, 2, 1)).astype(BF)
    wpack = np.concatenate(
        [
            np.asarray(Wq, dtype=np.float32).astype(BF).ravel(),
            np.asarray(Wk, dtype=np.float32).astype(BF).ravel(),
            np.asarray(Wv, dtype=np.float32).astype(BF).ravel(),
            np.asarray(Wo, dtype=np.float32).astype(BF).ravel(),
            np.asarray(bo, dtype=np.float32).astype(BF).ravel(),
        ]
    )
    maps = []
    for c in range(N_CORES):
        xin = np.empty(TOT, dtype=BF)
        xin[OFF_QT:OFF_KVT] = qT[S * c : S * (c + 1)].ravel()
        xin[OFF_KVT:OFF_W] = kvT[S * c : S * (c + 1)].ravel()
        xin[OFF_W:] = wpack
        maps.append({"xin": xin})
    return maps


def kernel(q_in, kv_in, Wq, Wk, Wv, Wo, bo):
    global _NC
    if _NC is None:
        _NC = build_nc()
    in_maps = make_in_maps(q_in, kv_in, Wq, Wk, Wv, Wo, bo)
    res = run_bass_kernel_spmd(_NC, in_maps, list(range(N_CORES))).results
    out = np.concatenate([res[c]["out"] for c in range(N_CORES)], axis=0)
    return out.reshape(4, 8, N, DIM)
